# revision 39
# baseline (speedup 1.0000x reference)
"""Multi-head self-attention TRN2 kernel (B=2, T=2048, E=1024, H=16, D=64).

Sharding: tensor-parallel over heads -- each of the 8 cores owns 2 heads.
Because the reference reshapes (B,H,T,D)->(B,T,E) with NO transpose, each
head's attention output maps to 128 complete contiguous rows of the
out_proj input, so the whole computation is embarrassingly parallel
across heads (no collectives).

Per-core pipeline (all matmuls fp16, accumulation fp32):
  1. qT/kT projection producing q^T,k^T in [(h,d), T] layout
     (heads stacked on partitions 0-63 / 64-127 -> row-tiled score matmuls).
  2. v in natural [T, d] layout, augmented with a ones column (gives the
     softmax denominator for free as column 64 of the P@v output).
  3. scores^T tiles [kj=128, qi] -> exp on ScalarE (no max subtraction:
     scores ~ N(0,1), exp is safe in fp32) -> P^T fp16.
  4. attn@v in natural-o orientation: o[qi,d] accumulated in PSUM with
     lhsT = P^T subtiles [128,128], rhs = v [128,65].  Streams d (65 wide)
     instead of qi (512 wide) -> half the PE cycles of the o^T form.
     The 4 qi-subtile accumulators of one head share one PSUM bank via a
     single-start accumulation group (zero regions are 2 KiB).
  5. normalize per-partition on DVE (denominator = col 64 -> [P,1]
     tensor_scalar, no partition broadcast), PE-transpose the normalized
     [128qi, 64d] tile to [64d, 128qi], DVE-copy into ofull.
  6. out_proj decomposed over j (the reshape mixing index) with the
     partition-shifted dup trick: 8 accumulating matmuls per 512 cols.
A junk-matmul warm-up burns the PE p-state ramp while prologue DMAs
stream in, so real matmuls start at full clock.
"""

import numpy as np

B, T, E, H, D = 2, 2048, 1024, 16, 64
N_CORES = 8
HL = H // N_CORES          # heads per core = 2
KP = E // 128              # 8 contraction partition-tiles
KT = T // 128              # 16 kj tiles
QC = T // 512              # 4 qi chunks of 512

_RUNNER = None


def _build_nc():
    import concourse.bacc as bacc
    import concourse.tile as tile
    import concourse.bass as bass
    import concourse.mybir as mybir

    fp32 = mybir.dt.float32
    f16 = mybir.dt.float16
    ADD = mybir.AluOpType.add
    MULT = mybir.AluOpType.mult
    EXP = mybir.ActivationFunctionType.Exp

    nc = bacc.Bacc("TRN2", target_bir_lowering=False, debug=False,
                   enable_asserts=True, num_devices=N_CORES)

    xt_d = nc.dram_tensor("xt", [E, B * T], f16, kind="ExternalInput").ap()
    wqk_d = nc.dram_tensor("wqk", [E, 4 * D], f16, kind="ExternalInput").ap()
    wv_d = nc.dram_tensor("wv", [E, 2 * (D + 1)], f16, kind="ExternalInput").ap()
    wout_d = nc.dram_tensor("wout", [128, 8 * E], f16, kind="ExternalInput").ap()
    bqk_d = nc.dram_tensor("bqk", [128, 2], fp32, kind="ExternalInput").ap()
    bv_d = nc.dram_tensor("bv", [128, 2 * (D + 1)], fp32, kind="ExternalInput").ap()
    bout_d = nc.dram_tensor("bout", [128, E], fp32, kind="ExternalInput").ap()
    ident_d = nc.dram_tensor("ident", [128, 128], f16, kind="ExternalInput").ap()
    y_d = nc.dram_tensor("y", [B, HL, 128, E], fp32, kind="ExternalOutput").ap()

    with tile.TileContext(nc) as tc:
        with (
            tc.tile_pool(name="const", bufs=1) as cpool,
            tc.tile_pool(name="ppool", bufs=12) as ppool,
            tc.tile_pool(name="npool", bufs=6) as npool,
            tc.tile_pool(name="ypool", bufs=3) as ypool,
            tc.tile_pool(name="ps_s", bufs=2, space=bass.MemorySpace.PSUM) as ps_s,
            tc.tile_pool(name="ps_o", bufs=1, space=bass.MemorySpace.PSUM) as ps_o,
            tc.tile_pool(name="ps_sm", bufs=2, space=bass.MemorySpace.PSUM) as ps_sm,
        ):
            # ---- constants / persistent tiles ----
            xt_sb = cpool.tile([128, KP, B * T], f16, tag="xt")
            wqk_sb = cpool.tile([128, KP, 4 * D], f16, tag="wqk")
            wv_sb = cpool.tile([128, KP, 2 * (D + 1)], f16, tag="wv")
            wout_sb = cpool.tile([128, 8, E], f16, tag="wout")
            bqk_sb = cpool.tile([128, 2], fp32, tag="bqk")
            bv_sb = cpool.tile([128, 2 * (D + 1)], fp32, tag="bv")
            bout_sb = cpool.tile([128, E], fp32, tag="bout")
            ident_sb = cpool.tile([128, 128], f16, tag="ident")
            warm_sb = cpool.tile([128, 128], f16, tag="warm")
            qkT = cpool.tile([128, B, 2, T], f16, tag="qkT")
            vaug = cpool.tile([128, B, KT, 2 * (D + 1)], f16, tag="vaug")
            ofull = cpool.tile([128, B, HL, T], f16, tag="ofull")

            xt_r = xt_d.rearrange("(a p) n -> p a n", p=128)
            wqk_r = wqk_d.rearrange("(a p) n -> p a n", p=128)
            wv_r = wv_d.rearrange("(a p) n -> p a n", p=128)

            # ---- prologue DMAs (SP queue; DMA engines serialize, so the
            # order is chosen to unblock the first qk-proj pieces fastest.
            # 256-column xt granularity keeps descriptor runs at 512B --
            # shorter runs pay a 2x DMA latency multiplier).
            nc.sync.dma_start(bqk_sb[:], bqk_d[:])
            nc.sync.dma_start(wqk_sb[:], wqk_r[:])
            nc.sync.dma_start(xt_sb[:, :, 0:256], xt_r[:, :, 0:256])
            nc.sync.dma_start(xt_sb[:, :, 256:512], xt_r[:, :, 256:512])
            nc.sync.dma_start(wv_sb[:], wv_r[:])
            nc.sync.dma_start(bv_sb[:], bv_d[:])
            # chunk 1 split in halves: cols 512:768 gate the k(0,1) filler
            # (kt 4-5) ~1.5us earlier than a single 512-col transfer would
            nc.sync.dma_start(xt_sb[:, :, 512:768], xt_r[:, :, 512:768])
            nc.sync.dma_start(xt_sb[:, :, 768:1024], xt_r[:, :, 768:1024])
            nc.sync.dma_start(ident_sb[:], ident_d[:])
            for cc in range(2, 8):
                nc.sync.dma_start(xt_sb[:, :, cc * 512:(cc + 1) * 512],
                                  xt_r[:, :, cc * 512:(cc + 1) * 512])

            # ---- PE warm-up: burn the p-state ramp on junk matmuls while
            # the prologue DMAs stream in.  warm_sb comes from a memset (no
            # DMA dependency) so the PE can start within ~0.3us. ----
            nc.vector.memset(warm_sb[:], 1.0)
            wps = ps_sm.tile([128, 128], fp32, tag="sm", name="warm")
            for i in range(26):
                nc.tensor.matmul(wps[:], warm_sb[:], warm_sb[:],
                                 start=(i == 0), stop=(i == 25))

            # ---- building blocks ----
            def proj_qk_m(b, n, m, c0=0, c1=512):
                ps = ps_sm.tile([128, c1 - c0], fp32, tag="sm", name="ps")
                for k in range(KP):
                    nc.tensor.matmul(
                        ps[:],
                        wqk_sb[:, k, m * 128:(m + 1) * 128],
                        xt_sb[:, k, b * T + n * 512 + c0: b * T + n * 512 + c1],
                        start=(k == 0), stop=(k == KP - 1),
                    )
                nc.vector.tensor_scalar(
                    qkT[:, b, m, n * 512 + c0:n * 512 + c1], ps[:],
                    bqk_sb[:, m:m + 1], None, op0=ADD,
                )

            def proj_v(b, r):
                vp = ps_sm.tile([128, 2 * (D + 1)], fp32, tag="sm", name="vp")
                for k in range(KP):
                    nc.tensor.matmul(
                        vp[:],
                        xt_sb[:, k, b * T + r * 128: b * T + (r + 1) * 128],
                        wv_sb[:, k, :],
                        start=(k == 0), stop=(k == KP - 1),
                    )
                nc.vector.tensor_tensor(
                    vaug[:, b, r, :], vp[:], bv_sb[:], op=ADD,
                )

            def sc(b, qc, kt):
                # scores^T tile for both heads, row-tiled on partitions
                # 0-63 / 64-127 so the two K=64 matmuls share the PE array
                S = ps_s.tile([128, 2 * 512], fp32, tag="S", name="S")
                for h in range(HL):
                    nc.tensor.matmul(
                        S[:, h * 512:(h + 1) * 512],
                        qkT[h * D:(h + 1) * D, b, 1, kt * 128:(kt + 1) * 128],
                        qkT[h * D:(h + 1) * D, b, 0, qc * 512:(qc + 1) * 512],
                        start=True, stop=True,
                    )
                return S

            def norm_mults(b, qc, os_, h):
                # per-partition normalize (denominator = col 64) on DVE.
                # Only these reads gate the o-accumulator bank reuse, so
                # they run at the round boundary; the transpose+copies are
                # deferred (drip-fed one per slice) to keep the PE queue
                # from head-of-line blocking on DVE copies.
                ot = os_[h]
                rc = npool.tile([128, 4, 1], fp32, tag="rc", name="rc")
                nc.vector.reciprocal(rc[:], ot[:, :, D:D + 1])
                fins = []
                for s in range(4):
                    onm = npool.tile([128, D], f16, tag="onm", name="onm",
                                     bufs=12)
                    nc.vector.tensor_scalar(
                        onm[:], ot[:, s, 0:D], rc[:, s, :], None, op0=MULT,
                    )
                    fins.append((b, qc, h, s, onm))
                return fins

            def norm_fin(b, qc, h, s, onm):
                # PE transpose [128qi,64d]->[64d,128qi], copy into ofull
                # rows 0-63 (DVE) plus a one-column-shifted copy into rows
                # 64-127 (Pool) -- the "dup" rows the paired out_proj lhsT
                # needs, without a DMA on the critical path.
                tp = ps_sm.tile([D, 128], f16, tag="sm", name="tp")
                nc.tensor.transpose(tp[:], onm[:], ident_sb[:])
                qt = qc * 4 + s
                nc.vector.tensor_copy(
                    ofull[0:D, b, h, qt * 128:(qt + 1) * 128], tp[:])
                # GPSIMD cannot read PSUM, so the shifted dup copy sources
                # from the SBUF range the DVE copy just wrote
                if qt == 0:
                    nc.gpsimd.tensor_copy(
                        ofull[D:128, b, h, 0:127],
                        ofull[0:D, b, h, 1:128])
                else:
                    nc.gpsimd.tensor_copy(
                        ofull[D:128, b, h, qt * 128 - 1:(qt + 1) * 128 - 1],
                        ofull[0:D, b, h, qt * 128:(qt + 1) * 128])

            def outproj_cols(b, h, c0, c1):
                of2 = ofull[:, b, h, :].rearrange("p (t j) -> p j t", j=16)
                yp = ps_sm.tile([128, c1 - c0], fp32, tag="sm", name="yp")
                for jj in range(8):
                    nc.tensor.matmul(
                        yp[:],
                        of2[:, 2 * jj, :],
                        wout_sb[:, jj, c0:c1],
                        start=(jj == 0), stop=(jj == 7),
                    )
                ys = ypool.tile([128, c1 - c0], fp32, tag="ys", name="ys")
                nc.vector.tensor_tensor(
                    ys[:], yp[:], bout_sb[:, c0:c1], op=ADD,
                )
                nc.sync.dma_start(y_d[b, h, :, c0:c1], ys[:])

            def outproj_n5(b, h, n5):
                outproj_cols(b, h, n5 * 512, (n5 + 1) * 512)

            # ---- filler schedule: slice index -> list of closures.
            # qk pieces that gate sc of slice s are placed at slot <= s-2
            # (sc(s) is emitted at slice s-1, after slot s-1's pieces).
            # Boundary slices (kt==0) carry a filler so the PE has work
            # while the previous round's normalize reads drain its PSUM
            # accumulator bank (ps_o bufs=1).
            def _kq(b, n, m, c0, c1):
                return lambda: proj_qk_m(b, n, m, c0, c1)

            def _q(b, n, half):
                return _kq(b, n, 0, half * 256, half * 256 + 256)

            def _k(b, n, half):
                return _kq(b, n, 1, half * 256, half * 256 + 256)

            def _v(b, r0):
                return lambda: [proj_v(b, r) for r in range(r0, r0 + 2)]

            SLOT = {
                # b0 r0: k half kt2-3, k chunks 1-3, v rows, q chunk 1
                0: [_kq(0, 0, 1, 256, 512)],
                1: [_v(0, 0)],
                2: [_k(0, 1, 0)], 3: [_k(0, 1, 1)],
                4: [_v(0, 2)], 5: [_v(0, 4)],
                6: [_k(0, 2, 0)], 7: [_k(0, 2, 1)],
                8: [_v(0, 6)],
                9: [_k(0, 3, 0)], 10: [_k(0, 3, 1)],
                11: [_v(0, 8)],
                12: [_v(0, 10)],
                13: [_q(0, 1, 0), _v(0, 12)],
                14: [_q(0, 1, 1)],
                15: [_v(0, 14)],
                # b0 r1
                16: [_q(0, 2, 0)], 17: [_q(0, 2, 1)],
                20: [_k(1, 0, 0)], 21: [_k(1, 0, 1)],
                24: [lambda: nc.sync.dma_start(wout_sb[:], wout_d[:])],
                26: [_q(0, 3, 0)], 27: [_q(0, 3, 1)],
                29: [lambda: nc.sync.dma_start(bout_sb[:], bout_d[:])],
                # b0 r2
                32: [_k(1, 1, 0)], 33: [_k(1, 1, 1)],
                36: [_q(1, 0, 0)], 37: [_q(1, 0, 1)],
                40: [_v(1, 0)], 42: [_v(1, 2)], 44: [_v(1, 4)], 46: [_v(1, 6)],
                # b0 r3
                48: [_k(1, 2, 0)], 49: [_k(1, 2, 1)],
                52: [_k(1, 3, 0)], 53: [_k(1, 3, 1)],
                55: [_v(1, 8)], 57: [_v(1, 10)], 59: [_v(1, 12)],
                61: [_q(1, 1, 0)], 62: [_q(1, 1, 1)],
                # b1 r0
                64: [_v(1, 14)],
                75: [_q(1, 2, 0)], 76: [_q(1, 2, 1)],
                # b1 r1
                89: [_q(1, 3, 0)], 90: [_q(1, 3, 1)],
            }
            # out_proj(b0) pieces go in the otherwise-empty b1 r2/r3 slack
            def _op0(h, c0):
                return lambda: outproj_cols(0, h, c0, c0 + 256)

            for _i, (_h, _c) in enumerate(
                    [(h, c) for h in range(HL) for c in range(0, E, 256)]):
                SLOT[97 + 2 * _i] = [_op0(_h, _c)]
            # slice -> (b, rows-complete watermark) applied after its slots
            V_SLOTS = {
                1: (0, 2), 4: (0, 4), 5: (0, 6), 8: (0, 8), 11: (0, 10),
                12: (0, 12), 13: (0, 14), 15: (0, 16),
                40: (1, 2), 42: (1, 4), 44: (1, 6), 46: (1, 8),
                55: (1, 10), 57: (1, 12), 59: (1, 14), 64: (1, 16),
            }

            def _emit_all():
                # prologue compute: q chunk 0 + k half kt0-1, interleaved
                # in DMA-arrival order (xt lands in 256-col pieces)
                proj_qk_m(0, 0, 0, 0, 256)
                proj_qk_m(0, 0, 1, 0, 256)
                _emit_rest()

            def _emit_rest():
                seq = [(b, qc, kt) for b in range(B) for qc in range(QC)
                       for kt in range(KT)]
                os_all = {}
                vaug_rows = {0: 0, 1: 0}   # vaug row tiles emitted so far
                pend_vmm = []              # [(b, qc, kt, P, s0, s1), ...]

                def flush_vmm(hold=None):
                    rest = []
                    for (vb, vqc, vkt, vP, s0, s1) in pend_vmm:
                        if (vb, vqc) == hold:
                            rest.append((vb, vqc, vkt, vP, s0, s1))
                        elif vkt < vaug_rows[vb]:
                            ot = os_all[(vb, vqc)]
                            hstr = 128 * (s1 - s0)
                            for h in range(HL):
                                for s in range(s0, s1):
                                    nc.tensor.matmul(
                                        ot[h][:, s, 0:D + 1],
                                        vP[:, h * hstr + (s - s0) * 128:
                                           h * hstr + (s - s0 + 1) * 128],
                                        vaug[:, vb, vkt,
                                             h * (D + 1):(h + 1) * (D + 1)],
                                        start=(vkt == 0 and s == 0),
                                        stop=(vkt == KT - 1 and s == 3),
                                        skip_group_check=True,
                                    )
                        else:
                            rest.append((vb, vqc, vkt, vP, s0, s1))
                    pend_vmm[:] = rest

                proj_qk_m(0, 0, 0, 256, 512)
                S = sc(*seq[0])

                pend_fin = []
                for i, (b, qc, kt) in enumerate(seq):
                    P = ppool.tile([128, 2 * 512], f16, tag="P")
                    nc.scalar.activation(P[:], S[:], EXP, scale=0.125)
                    # next scores FIRST so the ACT stream is never delayed
                    # behind a filler piece (slot pieces always have >= 2
                    # slices of margin before the sc that consumes them)
                    if i + 1 < len(seq):
                        S = sc(*seq[i + 1])
                    for piece in SLOT.get(i, ()):
                        piece()
                    if i in V_SLOTS:
                        vb, rows = V_SLOTS[i]
                        vaug_rows[vb] = rows
                    if kt == 0 and i > 0:
                        # previous round was normed at its own kt15 slice
                        # (below), so its o bank is already draining
                        pb, pqc = seq[i - 1][0], seq[i - 1][1]
                        assert (pb, pqc) not in os_all
                    if kt == 0:
                        os_all[(b, qc)] = [
                            ps_o.tile([128, 4, 128], fp32, tag=f"o{h}",
                                      name=f"o{h}")
                            for h in range(HL)]
                    pend_vmm.append((b, qc, kt, P, 0, 4))
                    flush_vmm()
                    # norm a round as soon as its last attn@v pass has been
                    # emitted -- one slice before the next round's kt0, so
                    # the o bank is free before the PE reaches the new
                    # round's first accumulating matmul
                    if kt == KT - 1 and not any(
                            x[0] == b and x[1] == qc for x in pend_vmm):
                        for h in range(HL):
                            pend_fin.extend(
                                norm_mults(b, qc, os_all[(b, qc)], h))
                        os_all.pop((b, qc))
                    if pend_fin:
                        norm_fin(*pend_fin.pop(0))
                flush_vmm()
                assert not pend_vmm
                # tail: the last round's mults were emitted in-loop at its
                # kt15 slice; drain the remaining fins, then 256-col
                # out_proj pieces so the PE matmuls of head h overlap the
                # copies of head h+1
                if (B - 1, QC - 1) in os_all:
                    os_last = os_all.pop((B - 1, QC - 1))
                    for h in range(HL):
                        pend_fin.extend(norm_mults(B - 1, QC - 1, os_last, h))
                for f in pend_fin:
                    norm_fin(*f)
                pend_fin = []
                for h in range(HL):
                    for c0 in range(0, E - 256, 256):
                        outproj_cols(1, h, c0, c0 + 256)
                # final 256 columns of both heads share one ys tile and one
                # store so only a single DMA chain is exposed at the end
                of2s = [ofull[:, 1, h, :].rearrange("p (t j) -> p j t", j=16)
                        for h in range(HL)]
                ys2 = ypool.tile([128, HL, 256], fp32, tag="ys2", name="ys2")
                for h in range(HL):
                    yp = ps_sm.tile([128, 256], fp32, tag="sm", name="yp")
                    for jj in range(8):
                        nc.tensor.matmul(
                            yp[:], of2s[h][:, 2 * jj, :],
                            wout_sb[:, jj, E - 256:E],
                            start=(jj == 0), stop=(jj == 7),
                        )
                    nc.vector.tensor_tensor(
                        ys2[:, h, :], yp[:], bout_sb[:, E - 256:E], op=ADD,
                    )
                nc.sync.dma_start(
                    y_d[1, :, :, E - 256:E].rearrange("h p c -> p h c"),
                    ys2[:])

            import os as _os
            _reps = int(_os.environ.get("KERNEL_EMIT_REPS", "1"))
            for _rep in range(_reps):
                _emit_all()

    nc.compile()
    return nc


def _get_runner():
    """Build + compile once; return a callable(in_maps) -> list of out dicts."""
    global _RUNNER
    if _RUNNER is not None:
        return _RUNNER

    import jax
    import concourse.mybir as mybir
    from concourse import bass2jax
    from jax.experimental.shard_map import shard_map
    from jax.sharding import Mesh, PartitionSpec

    nc = _build_nc()
    bass2jax.install_neuronx_cc_hook()

    partition_name = (nc.partition_id_tensor.name
                      if nc.partition_id_tensor else None)
    in_names, out_names, out_avals = [], [], []
    for alloc in nc.m.functions[0].allocations:
        if not isinstance(alloc, mybir.MemoryLocationSet):
            continue
        name = alloc.memorylocations[0].name
        if alloc.kind == "ExternalInput":
            if name != partition_name:
                in_names.append(name)
        elif alloc.kind == "ExternalOutput":
            out_names.append(name)
            out_avals.append(jax.core.ShapedArray(
                tuple(alloc.tensor_shape), mybir.dt.np(alloc.dtype)))

    n_params, n_outs = len(in_names), len(out_avals)
    all_names = in_names + out_names
    if partition_name is not None:
        all_names = all_names + [partition_name]

    def _body(*args):
        operands = list(args)
        if partition_name is not None:
            operands.append(bass2jax.partition_id_tensor())
        outs = bass2jax._bass_exec_p.bind(
            *operands,
            out_avals=tuple(out_avals),
            in_names=tuple(all_names),
            out_names=tuple(out_names),
            lowering_input_output_aliases=(),
            sim_require_finite=True,
            sim_require_nnan=True,
            nc=nc,
        )
        return tuple(outs)

    devices = jax.devices()[:N_CORES]
    mesh = Mesh(np.asarray(devices), ("core",))
    in_specs = (PartitionSpec("core"),) * (n_params + n_outs)
    out_specs = (PartitionSpec("core"),) * n_outs
    donate = tuple(range(n_params, n_params + n_outs))
    sharded = jax.jit(
        shard_map(_body, mesh=mesh, in_specs=in_specs, out_specs=out_specs,
                  check_rep=False),
        donate_argnums=donate, keep_unused=True,
    )

    def run(in_maps):
        concat_in = [
            np.concatenate([np.asarray(in_maps[c][nm]) for c in range(N_CORES)],
                           axis=0)
            for nm in in_names
        ]
        concat_zeros = [
            np.zeros((N_CORES * a.shape[0], *a.shape[1:]), a.dtype)
            for a in out_avals
        ]
        out_arrs = sharded(*concat_in, *concat_zeros)
        return [
            {nm: np.asarray(out_arrs[i]).reshape(N_CORES, *out_avals[i].shape)[c]
             for i, nm in enumerate(out_names)}
            for c in range(N_CORES)
        ]

    _RUNNER = run
    run._bench_parts = (sharded, mesh, in_names, out_names, out_avals,
                        n_params, _body)
    return run


def _make_bench(in_maps):
    """Device-resident benchmark closure: returns fn() that runs one
    execution with all inputs already on device (no donation)."""
    import jax
    from jax.experimental.shard_map import shard_map
    from jax.sharding import NamedSharding, PartitionSpec

    run = _get_runner()
    sharded, mesh, in_names, out_names, out_avals, n_params, _body = \
        run._bench_parts
    sh = NamedSharding(mesh, PartitionSpec("core"))

    nodonate = jax.jit(
        shard_map(_body, mesh=mesh,
                  in_specs=(PartitionSpec("core"),) * (n_params + len(out_avals)),
                  out_specs=(PartitionSpec("core"),) * len(out_avals),
                  check_rep=False),
        keep_unused=True,
    )
    concat_in = [
        np.concatenate([np.asarray(in_maps[c][nm]) for c in range(N_CORES)], axis=0)
        for nm in in_names
    ]
    concat_zeros = [
        np.zeros((N_CORES * a.shape[0], *a.shape[1:]), a.dtype) for a in out_avals
    ]
    dev_args = [jax.device_put(a, sh) for a in concat_in + concat_zeros]
    for a in dev_args:
        a.block_until_ready()

    def bench_once():
        outs = nodonate(*dev_args)
        for o in outs:
            o.block_until_ready()
        return outs

    def make_bench_k(k):
        n_in = len(in_names)

        def _body_k(*args):
            ins = list(args[:n_in])
            zs = list(args[n_in:])
            for _ in range(k):
                zs = list(_body(*ins, *zs))
            return tuple(zs)

        jk = jax.jit(
            shard_map(_body_k, mesh=mesh,
                      in_specs=(PartitionSpec("core"),) * len(dev_args),
                      out_specs=(PartitionSpec("core"),) * len(out_avals),
                      check_rep=False),
            keep_unused=True,
        )

        def run_k():
            outs = jk(*dev_args)
            for o in outs:
                o.block_until_ready()
            return outs

        return run_k

    bench_once.make_bench_k = make_bench_k
    bench_once.nodonate = nodonate
    bench_once.dev_args = dev_args
    return bench_once


def _prep_in_maps(x, W_qkv, b_qkv, W_out, b_out):
    f16 = np.float16
    xt = np.ascontiguousarray(
        x.reshape(B * T, E).T).astype(f16)                     # [E, B*T]
    wout = np.ascontiguousarray(
        W_out.reshape(8, 128, E).transpose(1, 0, 2).reshape(128, 8 * E)).astype(f16)
    bout = np.ascontiguousarray(
        np.broadcast_to(b_out.astype(np.float32)[None, :], (128, E)))
    ident = np.eye(128, dtype=f16)

    in_maps = []
    for c in range(N_CORES):
        hs = [HL * c + i for i in range(HL)]
        qcols = np.concatenate(
            [W_qkv[:, 0 * E + h * D:0 * E + (h + 1) * D] for h in hs], axis=1)
        kcols = np.concatenate(
            [W_qkv[:, 1 * E + h * D:1 * E + (h + 1) * D] for h in hs], axis=1)
        wqk = np.ascontiguousarray(
            np.concatenate([qcols, kcols], axis=1)).astype(f16)  # [E, 256]
        zcol = np.zeros((E, 1), np.float32)
        wv = np.ascontiguousarray(np.concatenate(
            [arr for h in hs
             for arr in (W_qkv[:, 2 * E + h * D:2 * E + (h + 1) * D], zcol)],
            axis=1)).astype(f16)                                 # [E, 130]
        bq = np.concatenate([b_qkv[0 * E + h * D:0 * E + (h + 1) * D] for h in hs])
        bk = np.concatenate([b_qkv[1 * E + h * D:1 * E + (h + 1) * D] for h in hs])
        bqk = np.ascontiguousarray(
            np.stack([bq, bk], axis=1)).astype(np.float32)      # [128, 2]
        one = np.ones(1, np.float32)
        bvv = np.concatenate(
            [a for h in hs
             for a in (b_qkv[2 * E + h * D:2 * E + (h + 1) * D], one)])
        bv = np.ascontiguousarray(
            np.broadcast_to(bvv.astype(np.float32)[None, :], (128, 2 * (D + 1))))
        in_maps.append({
            "xt": xt, "wqk": wqk, "wv": wv, "wout": wout,
            "bqk": bqk, "bv": bv, "bout": bout, "ident": ident,
        })
    return in_maps


def kernel(x, W_qkv, b_qkv, W_out, b_out):
    x = np.asarray(x, dtype=np.float32)
    W_qkv = np.asarray(W_qkv, dtype=np.float32)
    b_qkv = np.asarray(b_qkv, dtype=np.float32)
    W_out = np.asarray(W_out, dtype=np.float32)
    b_out = np.asarray(b_out, dtype=np.float32)

    run = _get_runner()
    in_maps = _prep_in_maps(x, W_qkv, b_qkv, W_out, b_out)
    results = run(in_maps)

    out = np.empty((B, T, E), np.float32)
    for c in range(N_CORES):
        y = results[c]["y"]          # [B, HL, 128, E]
        for hl in range(HL):
            hg = HL * c + hl
            out[:, hg * 128:(hg + 1) * 128, :] = y[:, hl]
    return out


# revision 41
# speedup vs baseline: 1.0088x; 1.0088x over previous
"""Multi-head self-attention TRN2 kernel (B=2, T=2048, E=1024, H=16, D=64).

Sharding: tensor-parallel over heads -- each of the 8 cores owns 2 heads.
Because the reference reshapes (B,H,T,D)->(B,T,E) with NO transpose, each
head's attention output maps to 128 complete contiguous rows of the
out_proj input, so the whole computation is embarrassingly parallel
across heads (no collectives).

Per-core pipeline (all matmuls fp16, accumulation fp32):
  1. qT/kT projection producing q^T,k^T in [(h,d), T] layout
     (heads stacked on partitions 0-63 / 64-127 -> row-tiled score matmuls).
  2. v in natural [T, d] layout, augmented with a ones column (gives the
     softmax denominator for free as column 64 of the P@v output).
  3. scores^T tiles [kj=128, qi] -> exp on ScalarE (no max subtraction:
     scores ~ N(0,1), exp is safe in fp32) -> P^T fp16.
  4. attn@v in natural-o orientation: o[qi,d] accumulated in PSUM with
     lhsT = P^T subtiles [128,128], rhs = v [128,65].  Streams d (65 wide)
     instead of qi (512 wide) -> half the PE cycles of the o^T form.
     The 4 qi-subtile accumulators of one head share one PSUM bank via a
     single-start accumulation group (zero regions are 2 KiB).
  5. normalize per-partition on DVE (denominator = col 64 -> [P,1]
     tensor_scalar, no partition broadcast), PE-transpose the normalized
     [128qi, 64d] tile to [64d, 128qi], DVE-copy into ofull.
  6. out_proj decomposed over j (the reshape mixing index) with the
     partition-shifted dup trick: 8 accumulating matmuls per 512 cols.
A junk-matmul warm-up burns the PE p-state ramp while prologue DMAs
stream in, so real matmuls start at full clock.
"""

import numpy as np

B, T, E, H, D = 2, 2048, 1024, 16, 64
N_CORES = 8
HL = H // N_CORES          # heads per core = 2
KP = E // 128              # 8 contraction partition-tiles
KT = T // 128              # 16 kj tiles
QC = T // 512              # 4 qi chunks of 512

_RUNNER = None


def _build_nc():
    import concourse.bacc as bacc
    import concourse.tile as tile
    import concourse.bass as bass
    import concourse.mybir as mybir

    fp32 = mybir.dt.float32
    f16 = mybir.dt.float16
    ADD = mybir.AluOpType.add
    MULT = mybir.AluOpType.mult
    EXP = mybir.ActivationFunctionType.Exp

    nc = bacc.Bacc("TRN2", target_bir_lowering=False, debug=False,
                   enable_asserts=True, num_devices=N_CORES)

    xt_d = nc.dram_tensor("xt", [E, B * T], f16, kind="ExternalInput").ap()
    wqk_d = nc.dram_tensor("wqk", [E, 4 * D], f16, kind="ExternalInput").ap()
    wv_d = nc.dram_tensor("wv", [E, 2 * (D + 1)], f16, kind="ExternalInput").ap()
    wout_d = nc.dram_tensor("wout", [128, 8 * E], f16, kind="ExternalInput").ap()
    bqk_d = nc.dram_tensor("bqk", [128, 2], fp32, kind="ExternalInput").ap()
    bv_d = nc.dram_tensor("bv", [128, 2 * (D + 1)], fp32, kind="ExternalInput").ap()
    bout_d = nc.dram_tensor("bout", [128, E], fp32, kind="ExternalInput").ap()
    ident_d = nc.dram_tensor("ident", [128, 128], f16, kind="ExternalInput").ap()
    y_d = nc.dram_tensor("y", [B, HL, 128, E], fp32, kind="ExternalOutput").ap()

    with tile.TileContext(nc) as tc:
        with (
            tc.tile_pool(name="const", bufs=1) as cpool,
            tc.tile_pool(name="ppool", bufs=12) as ppool,
            tc.tile_pool(name="npool", bufs=6) as npool,
            tc.tile_pool(name="ypool", bufs=4) as ypool,
            tc.tile_pool(name="ps_s", bufs=2, space=bass.MemorySpace.PSUM) as ps_s,
            tc.tile_pool(name="ps_o", bufs=1, space=bass.MemorySpace.PSUM) as ps_o,
            tc.tile_pool(name="ps_sm", bufs=2, space=bass.MemorySpace.PSUM) as ps_sm,
        ):
            # ---- constants / persistent tiles ----
            xt_sb = cpool.tile([128, KP, B * T], f16, tag="xt")
            wqk_sb = cpool.tile([128, KP, 4 * D], f16, tag="wqk")
            wv_sb = cpool.tile([128, KP, 2 * (D + 1)], f16, tag="wv")
            wout_sb = cpool.tile([128, 8, E], f16, tag="wout")
            bqk_sb = cpool.tile([128, 2], fp32, tag="bqk")
            bv_sb = cpool.tile([128, 2 * (D + 1)], fp32, tag="bv")
            bout_sb = cpool.tile([128, E], fp32, tag="bout")
            ident_sb = cpool.tile([128, 128], f16, tag="ident")
            warm_sb = cpool.tile([128, 128], f16, tag="warm")
            qkT = cpool.tile([128, B, 2, T], f16, tag="qkT")
            vaug = cpool.tile([128, B, KT, 2 * (D + 1)], f16, tag="vaug")
            ofull = cpool.tile([128, B, HL, T], f16, tag="ofull")

            xt_r = xt_d.rearrange("(a p) n -> p a n", p=128)
            wqk_r = wqk_d.rearrange("(a p) n -> p a n", p=128)
            wv_r = wv_d.rearrange("(a p) n -> p a n", p=128)

            # ---- prologue DMAs (SP queue; DMA engines serialize, so the
            # order is chosen to unblock the first qk-proj pieces fastest.
            # 256-column xt granularity keeps descriptor runs at 512B --
            # shorter runs pay a 2x DMA latency multiplier).
            nc.sync.dma_start(bqk_sb[:], bqk_d[:])
            nc.sync.dma_start(wqk_sb[:], wqk_r[:])
            nc.sync.dma_start(xt_sb[:, :, 0:256], xt_r[:, :, 0:256])
            nc.sync.dma_start(xt_sb[:, :, 256:512], xt_r[:, :, 256:512])
            nc.sync.dma_start(wv_sb[:], wv_r[:])
            nc.sync.dma_start(bv_sb[:], bv_d[:])
            # chunk 1 split in halves: cols 512:768 gate the k(0,1) filler
            # (kt 4-5) ~1.5us earlier than a single 512-col transfer would
            nc.sync.dma_start(xt_sb[:, :, 512:768], xt_r[:, :, 512:768])
            nc.sync.dma_start(xt_sb[:, :, 768:1024], xt_r[:, :, 768:1024])
            nc.sync.dma_start(ident_sb[:], ident_d[:])
            for cc in range(2, 8):
                nc.sync.dma_start(xt_sb[:, :, cc * 512:(cc + 1) * 512],
                                  xt_r[:, :, cc * 512:(cc + 1) * 512])

            # ---- PE warm-up: burn the p-state ramp on junk matmuls while
            # the prologue DMAs stream in.  warm_sb comes from a memset (no
            # DMA dependency) so the PE can start within ~0.3us. ----
            nc.vector.memset(warm_sb[:], 1.0)
            wps = ps_sm.tile([128, 128], fp32, tag="sm", name="warm")
            for i in range(26):
                nc.tensor.matmul(wps[:], warm_sb[:], warm_sb[:],
                                 start=(i == 0), stop=(i == 25))

            # ---- building blocks ----
            def proj_qk_m(b, n, m, c0=0, c1=512):
                ps = ps_sm.tile([128, c1 - c0], fp32, tag="sm", name="ps")
                for k in range(KP):
                    nc.tensor.matmul(
                        ps[:],
                        wqk_sb[:, k, m * 128:(m + 1) * 128],
                        xt_sb[:, k, b * T + n * 512 + c0: b * T + n * 512 + c1],
                        start=(k == 0), stop=(k == KP - 1),
                    )
                nc.vector.tensor_scalar(
                    qkT[:, b, m, n * 512 + c0:n * 512 + c1], ps[:],
                    bqk_sb[:, m:m + 1], None, op0=ADD,
                )

            def proj_v(b, r):
                vp = ps_sm.tile([128, 2 * (D + 1)], fp32, tag="sm", name="vp")
                for k in range(KP):
                    nc.tensor.matmul(
                        vp[:],
                        xt_sb[:, k, b * T + r * 128: b * T + (r + 1) * 128],
                        wv_sb[:, k, :],
                        start=(k == 0), stop=(k == KP - 1),
                    )
                nc.vector.tensor_tensor(
                    vaug[:, b, r, :], vp[:], bv_sb[:], op=ADD,
                )

            def sc(b, qc, kt):
                # scores^T tile for both heads, row-tiled on partitions
                # 0-63 / 64-127 so the two K=64 matmuls share the PE array
                S = ps_s.tile([128, 2 * 512], fp32, tag="S", name="S")
                for h in range(HL):
                    nc.tensor.matmul(
                        S[:, h * 512:(h + 1) * 512],
                        qkT[h * D:(h + 1) * D, b, 1, kt * 128:(kt + 1) * 128],
                        qkT[h * D:(h + 1) * D, b, 0, qc * 512:(qc + 1) * 512],
                        start=True, stop=True,
                    )
                return S

            def norm_mults(b, qc, os_, h):
                # per-partition normalize (denominator = col 64) on DVE.
                # Only these reads gate the o-accumulator bank reuse, so
                # they run at the round boundary; the transpose+copies are
                # deferred (drip-fed one per slice) to keep the PE queue
                # from head-of-line blocking on DVE copies.
                ot = os_[h]
                rc = npool.tile([128, 4, 1], fp32, tag="rc", name="rc")
                nc.vector.reciprocal(rc[:], ot[:, :, D:D + 1])
                fins = []
                for s in range(4):
                    onm = npool.tile([128, D], f16, tag="onm", name="onm",
                                     bufs=12)
                    nc.vector.tensor_scalar(
                        onm[:], ot[:, s, 0:D], rc[:, s, :], None, op0=MULT,
                    )
                    fins.append((b, qc, h, s, onm))
                return fins

            def norm_fin(b, qc, h, s, onm):
                # PE transpose [128qi,64d]->[64d,128qi], copy into ofull
                # rows 0-63 (DVE) plus a one-column-shifted copy into rows
                # 64-127 (Pool) -- the "dup" rows the paired out_proj lhsT
                # needs, without a DMA on the critical path.
                tp = ps_sm.tile([D, 128], f16, tag="sm", name="tp")
                nc.tensor.transpose(tp[:], onm[:], ident_sb[:])
                qt = qc * 4 + s
                nc.vector.tensor_copy(
                    ofull[0:D, b, h, qt * 128:(qt + 1) * 128], tp[:])
                # GPSIMD cannot read PSUM, so the shifted dup copy sources
                # from the SBUF range the DVE copy just wrote
                if qt == 0:
                    nc.gpsimd.tensor_copy(
                        ofull[D:128, b, h, 0:127],
                        ofull[0:D, b, h, 1:128])
                else:
                    nc.gpsimd.tensor_copy(
                        ofull[D:128, b, h, qt * 128 - 1:(qt + 1) * 128 - 1],
                        ofull[0:D, b, h, qt * 128:(qt + 1) * 128])

            def outproj_cols(b, h, c0, c1):
                of2 = ofull[:, b, h, :].rearrange("p (t j) -> p j t", j=16)
                yp = ps_sm.tile([128, c1 - c0], fp32, tag="sm", name="yp")
                for jj in range(8):
                    nc.tensor.matmul(
                        yp[:],
                        of2[:, 2 * jj, :],
                        wout_sb[:, jj, c0:c1],
                        start=(jj == 0), stop=(jj == 7),
                    )
                ys = ypool.tile([128, c1 - c0], fp32, tag="ys", name="ys")
                nc.vector.tensor_tensor(
                    ys[:], yp[:], bout_sb[:, c0:c1], op=ADD,
                )
                nc.sync.dma_start(y_d[b, h, :, c0:c1], ys[:])

            def outproj_n5(b, h, n5):
                outproj_cols(b, h, n5 * 512, (n5 + 1) * 512)

            # ---- filler schedule: slice index -> list of closures.
            # qk pieces that gate sc of slice s are placed at slot <= s-2
            # (sc(s) is emitted at slice s-1, after slot s-1's pieces).
            # Boundary slices (kt==0) carry a filler so the PE has work
            # while the previous round's normalize reads drain its PSUM
            # accumulator bank (ps_o bufs=1).
            def _kq(b, n, m, c0, c1):
                return lambda: proj_qk_m(b, n, m, c0, c1)

            def _q(b, n, half):
                return _kq(b, n, 0, half * 256, half * 256 + 256)

            def _k(b, n, half):
                return _kq(b, n, 1, half * 256, half * 256 + 256)

            def _v(b, r0):
                return lambda: [proj_v(b, r) for r in range(r0, r0 + 2)]

            SLOT = {
                # b0 r0: k half kt2-3, k chunks 1-3, v rows, q chunk 1
                0: [_kq(0, 0, 1, 256, 512)],
                1: [_v(0, 0)],
                2: [_k(0, 1, 0)], 3: [_k(0, 1, 1)],
                4: [_v(0, 2)], 5: [_v(0, 4)],
                6: [_k(0, 2, 0)], 7: [_k(0, 2, 1)],
                8: [_v(0, 6)],
                9: [_k(0, 3, 0)], 10: [_k(0, 3, 1)],
                11: [_v(0, 8)],
                12: [_v(0, 10)],
                13: [_q(0, 1, 0), _v(0, 12)],
                14: [_q(0, 1, 1)],
                15: [_v(0, 14)],
                # b0 r1
                16: [_q(0, 2, 0)], 17: [_q(0, 2, 1)],
                20: [_k(1, 0, 0)], 21: [_k(1, 0, 1)],
                24: [lambda: nc.sync.dma_start(wout_sb[:], wout_d[:])],
                26: [_q(0, 3, 0)], 27: [_q(0, 3, 1)],
                29: [lambda: nc.sync.dma_start(bout_sb[:], bout_d[:])],
                # b0 r2
                32: [_k(1, 1, 0)], 33: [_k(1, 1, 1)],
                36: [_q(1, 0, 0)], 37: [_q(1, 0, 1)],
                40: [_v(1, 0)], 42: [_v(1, 2)], 44: [_v(1, 4)], 46: [_v(1, 6)],
                # b0 r3
                48: [_k(1, 2, 0)], 49: [_k(1, 2, 1)],
                52: [_k(1, 3, 0)], 53: [_k(1, 3, 1)],
                55: [_v(1, 8)], 57: [_v(1, 10)], 59: [_v(1, 12)],
                61: [_q(1, 1, 0)], 62: [_q(1, 1, 1)],
                # b1 r0
                64: [_v(1, 14)],
                75: [_q(1, 2, 0)], 76: [_q(1, 2, 1)],
                # b1 r1
                89: [_q(1, 3, 0)], 90: [_q(1, 3, 1)],
            }
            # out_proj(b0) pieces go in the otherwise-empty b1 r2/r3 slack
            def _op0(h, c0):
                return lambda: outproj_cols(0, h, c0, c0 + 256)

            for _i, (_h, _c) in enumerate(
                    [(h, c) for h in range(HL) for c in range(0, E, 256)]):
                SLOT[97 + 2 * _i] = [_op0(_h, _c)]
            # slice -> (b, rows-complete watermark) applied after its slots
            V_SLOTS = {
                1: (0, 2), 4: (0, 4), 5: (0, 6), 8: (0, 8), 11: (0, 10),
                12: (0, 12), 13: (0, 14), 15: (0, 16),
                40: (1, 2), 42: (1, 4), 44: (1, 6), 46: (1, 8),
                55: (1, 10), 57: (1, 12), 59: (1, 14), 64: (1, 16),
            }

            def _emit_all():
                # prologue compute: q chunk 0 + k half kt0-1, interleaved
                # in DMA-arrival order (xt lands in 256-col pieces)
                proj_qk_m(0, 0, 0, 0, 256)
                proj_qk_m(0, 0, 1, 0, 256)
                _emit_rest()

            def _emit_rest():
                seq = [(b, qc, kt) for b in range(B) for qc in range(QC)
                       for kt in range(KT)]
                os_all = {}
                vaug_rows = {0: 0, 1: 0}   # vaug row tiles emitted so far
                pend_vmm = []              # [(b, qc, kt, P, s0, s1), ...]

                def flush_vmm(hold=None):
                    rest = []
                    for (vb, vqc, vkt, vP, s0, s1) in pend_vmm:
                        if (vb, vqc) == hold:
                            rest.append((vb, vqc, vkt, vP, s0, s1))
                        elif vkt < vaug_rows[vb]:
                            ot = os_all[(vb, vqc)]
                            hstr = 128 * (s1 - s0)
                            for h in range(HL):
                                for s in range(s0, s1):
                                    nc.tensor.matmul(
                                        ot[h][:, s, 0:D + 1],
                                        vP[:, h * hstr + (s - s0) * 128:
                                           h * hstr + (s - s0 + 1) * 128],
                                        vaug[:, vb, vkt,
                                             h * (D + 1):(h + 1) * (D + 1)],
                                        start=(vkt == 0 and s == 0),
                                        stop=(vkt == KT - 1 and s == 3),
                                        skip_group_check=True,
                                    )
                        else:
                            rest.append((vb, vqc, vkt, vP, s0, s1))
                    pend_vmm[:] = rest

                proj_qk_m(0, 0, 0, 256, 512)
                S = sc(*seq[0])

                pend_fin = []
                for i, (b, qc, kt) in enumerate(seq):
                    P = ppool.tile([128, 2 * 512], f16, tag="P")
                    nc.scalar.activation(P[:], S[:], EXP, scale=0.125)
                    # next scores FIRST so the ACT stream is never delayed
                    # behind a filler piece (slot pieces always have >= 2
                    # slices of margin before the sc that consumes them)
                    if i + 1 < len(seq):
                        S = sc(*seq[i + 1])
                    for piece in SLOT.get(i, ()):
                        piece()
                    if i in V_SLOTS:
                        vb, rows = V_SLOTS[i]
                        vaug_rows[vb] = rows
                    if kt == 0 and i > 0:
                        pb, pqc = seq[i - 1][0], seq[i - 1][1]
                        assert not any(x[0] == pb and x[1] == pqc
                                       for x in pend_vmm)
                        for h in range(HL):
                            pend_fin.extend(
                                norm_mults(pb, pqc, os_all[(pb, pqc)], h))
                        os_all.pop((pb, pqc))
                    hold = None
                    if kt == 0:
                        os_all[(b, qc)] = [
                            ps_o.tile([128, 4, 128], fp32, tag=f"o{h}",
                                      name=f"o{h}")
                            for h in range(HL)]
                        # defer this round's first attn@v by one slice so
                        # the PE isn't head-of-line blocked on the previous
                        # round's normalize reads draining the o bank
                        hold = (b, qc)
                    pend_vmm.append((b, qc, kt, P, 0, 4))
                    flush_vmm(hold)
                    if pend_fin:
                        norm_fin(*pend_fin.pop(0))
                flush_vmm()
                assert not pend_vmm
                # tail: normalize mults first (DVE), then fins and 256-col
                # out_proj pieces interleaved so the PE matmuls of head h
                # overlap the copies of head h+1
                os_last = os_all.pop((B - 1, QC - 1))
                for h in range(HL):
                    pend_fin.extend(norm_mults(B - 1, QC - 1, os_last, h))
                for f in pend_fin:
                    norm_fin(*f)
                pend_fin = []
                for h in range(HL):
                    for c0 in range(0, E - 256, 256):
                        outproj_cols(1, h, c0, c0 + 256)
                # final 256 columns of both heads share one ys tile and one
                # store so only a single DMA chain is exposed at the end
                of2s = [ofull[:, 1, h, :].rearrange("p (t j) -> p j t", j=16)
                        for h in range(HL)]
                ys2 = ypool.tile([128, HL, 256], fp32, tag="ys2", name="ys2")
                for h in range(HL):
                    yp = ps_sm.tile([128, 256], fp32, tag="sm", name="yp")
                    for jj in range(8):
                        nc.tensor.matmul(
                            yp[:], of2s[h][:, 2 * jj, :],
                            wout_sb[:, jj, E - 256:E],
                            start=(jj == 0), stop=(jj == 7),
                        )
                    nc.vector.tensor_tensor(
                        ys2[:, h, :], yp[:], bout_sb[:, E - 256:E], op=ADD,
                    )
                nc.sync.dma_start(
                    y_d[1, :, :, E - 256:E].rearrange("h p c -> p h c"),
                    ys2[:])

            import os as _os
            _reps = int(_os.environ.get("KERNEL_EMIT_REPS", "1"))
            for _rep in range(_reps):
                _emit_all()

    nc.compile()
    return nc


def _get_runner():
    """Build + compile once; return a callable(in_maps) -> list of out dicts."""
    global _RUNNER
    if _RUNNER is not None:
        return _RUNNER

    import jax
    import concourse.mybir as mybir
    from concourse import bass2jax
    from jax.experimental.shard_map import shard_map
    from jax.sharding import Mesh, PartitionSpec

    nc = _build_nc()
    bass2jax.install_neuronx_cc_hook()

    partition_name = (nc.partition_id_tensor.name
                      if nc.partition_id_tensor else None)
    in_names, out_names, out_avals = [], [], []
    for alloc in nc.m.functions[0].allocations:
        if not isinstance(alloc, mybir.MemoryLocationSet):
            continue
        name = alloc.memorylocations[0].name
        if alloc.kind == "ExternalInput":
            if name != partition_name:
                in_names.append(name)
        elif alloc.kind == "ExternalOutput":
            out_names.append(name)
            out_avals.append(jax.core.ShapedArray(
                tuple(alloc.tensor_shape), mybir.dt.np(alloc.dtype)))

    n_params, n_outs = len(in_names), len(out_avals)
    all_names = in_names + out_names
    if partition_name is not None:
        all_names = all_names + [partition_name]

    def _body(*args):
        operands = list(args)
        if partition_name is not None:
            operands.append(bass2jax.partition_id_tensor())
        outs = bass2jax._bass_exec_p.bind(
            *operands,
            out_avals=tuple(out_avals),
            in_names=tuple(all_names),
            out_names=tuple(out_names),
            lowering_input_output_aliases=(),
            sim_require_finite=True,
            sim_require_nnan=True,
            nc=nc,
        )
        return tuple(outs)

    devices = jax.devices()[:N_CORES]
    mesh = Mesh(np.asarray(devices), ("core",))
    in_specs = (PartitionSpec("core"),) * (n_params + n_outs)
    out_specs = (PartitionSpec("core"),) * n_outs
    donate = tuple(range(n_params, n_params + n_outs))
    sharded = jax.jit(
        shard_map(_body, mesh=mesh, in_specs=in_specs, out_specs=out_specs,
                  check_rep=False),
        donate_argnums=donate, keep_unused=True,
    )

    def run(in_maps):
        concat_in = [
            np.concatenate([np.asarray(in_maps[c][nm]) for c in range(N_CORES)],
                           axis=0)
            for nm in in_names
        ]
        concat_zeros = [
            np.zeros((N_CORES * a.shape[0], *a.shape[1:]), a.dtype)
            for a in out_avals
        ]
        out_arrs = sharded(*concat_in, *concat_zeros)
        return [
            {nm: np.asarray(out_arrs[i]).reshape(N_CORES, *out_avals[i].shape)[c]
             for i, nm in enumerate(out_names)}
            for c in range(N_CORES)
        ]

    _RUNNER = run
    run._bench_parts = (sharded, mesh, in_names, out_names, out_avals,
                        n_params, _body)
    return run


def _make_bench(in_maps):
    """Device-resident benchmark closure: returns fn() that runs one
    execution with all inputs already on device (no donation)."""
    import jax
    from jax.experimental.shard_map import shard_map
    from jax.sharding import NamedSharding, PartitionSpec

    run = _get_runner()
    sharded, mesh, in_names, out_names, out_avals, n_params, _body = \
        run._bench_parts
    sh = NamedSharding(mesh, PartitionSpec("core"))

    nodonate = jax.jit(
        shard_map(_body, mesh=mesh,
                  in_specs=(PartitionSpec("core"),) * (n_params + len(out_avals)),
                  out_specs=(PartitionSpec("core"),) * len(out_avals),
                  check_rep=False),
        keep_unused=True,
    )
    concat_in = [
        np.concatenate([np.asarray(in_maps[c][nm]) for c in range(N_CORES)], axis=0)
        for nm in in_names
    ]
    concat_zeros = [
        np.zeros((N_CORES * a.shape[0], *a.shape[1:]), a.dtype) for a in out_avals
    ]
    dev_args = [jax.device_put(a, sh) for a in concat_in + concat_zeros]
    for a in dev_args:
        a.block_until_ready()

    def bench_once():
        outs = nodonate(*dev_args)
        for o in outs:
            o.block_until_ready()
        return outs

    def make_bench_k(k):
        n_in = len(in_names)

        def _body_k(*args):
            ins = list(args[:n_in])
            zs = list(args[n_in:])
            for _ in range(k):
                zs = list(_body(*ins, *zs))
            return tuple(zs)

        jk = jax.jit(
            shard_map(_body_k, mesh=mesh,
                      in_specs=(PartitionSpec("core"),) * len(dev_args),
                      out_specs=(PartitionSpec("core"),) * len(out_avals),
                      check_rep=False),
            keep_unused=True,
        )

        def run_k():
            outs = jk(*dev_args)
            for o in outs:
                o.block_until_ready()
            return outs

        return run_k

    bench_once.make_bench_k = make_bench_k
    bench_once.nodonate = nodonate
    bench_once.dev_args = dev_args
    return bench_once


def _prep_in_maps(x, W_qkv, b_qkv, W_out, b_out):
    f16 = np.float16
    xt = np.ascontiguousarray(
        x.reshape(B * T, E).T).astype(f16)                     # [E, B*T]
    wout = np.ascontiguousarray(
        W_out.reshape(8, 128, E).transpose(1, 0, 2).reshape(128, 8 * E)).astype(f16)
    bout = np.ascontiguousarray(
        np.broadcast_to(b_out.astype(np.float32)[None, :], (128, E)))
    ident = np.eye(128, dtype=f16)

    in_maps = []
    for c in range(N_CORES):
        hs = [HL * c + i for i in range(HL)]
        qcols = np.concatenate(
            [W_qkv[:, 0 * E + h * D:0 * E + (h + 1) * D] for h in hs], axis=1)
        kcols = np.concatenate(
            [W_qkv[:, 1 * E + h * D:1 * E + (h + 1) * D] for h in hs], axis=1)
        wqk = np.ascontiguousarray(
            np.concatenate([qcols, kcols], axis=1)).astype(f16)  # [E, 256]
        zcol = np.zeros((E, 1), np.float32)
        wv = np.ascontiguousarray(np.concatenate(
            [arr for h in hs
             for arr in (W_qkv[:, 2 * E + h * D:2 * E + (h + 1) * D], zcol)],
            axis=1)).astype(f16)                                 # [E, 130]
        bq = np.concatenate([b_qkv[0 * E + h * D:0 * E + (h + 1) * D] for h in hs])
        bk = np.concatenate([b_qkv[1 * E + h * D:1 * E + (h + 1) * D] for h in hs])
        bqk = np.ascontiguousarray(
            np.stack([bq, bk], axis=1)).astype(np.float32)      # [128, 2]
        one = np.ones(1, np.float32)
        bvv = np.concatenate(
            [a for h in hs
             for a in (b_qkv[2 * E + h * D:2 * E + (h + 1) * D], one)])
        bv = np.ascontiguousarray(
            np.broadcast_to(bvv.astype(np.float32)[None, :], (128, 2 * (D + 1))))
        in_maps.append({
            "xt": xt, "wqk": wqk, "wv": wv, "wout": wout,
            "bqk": bqk, "bv": bv, "bout": bout, "ident": ident,
        })
    return in_maps


def kernel(x, W_qkv, b_qkv, W_out, b_out):
    x = np.asarray(x, dtype=np.float32)
    W_qkv = np.asarray(W_qkv, dtype=np.float32)
    b_qkv = np.asarray(b_qkv, dtype=np.float32)
    W_out = np.asarray(W_out, dtype=np.float32)
    b_out = np.asarray(b_out, dtype=np.float32)

    run = _get_runner()
    in_maps = _prep_in_maps(x, W_qkv, b_qkv, W_out, b_out)
    results = run(in_maps)

    out = np.empty((B, T, E), np.float32)
    for c in range(N_CORES):
        y = results[c]["y"]          # [B, HL, 128, E]
        for hl in range(HL):
            hg = HL * c + hl
            out[:, hg * 128:(hg + 1) * 128, :] = y[:, hl]
    return out
